# revision 1
# baseline (speedup 1.0000x reference)
"""Block-sparse self-attention Trainium2 kernel (8 NeuronCores).

Strategy
--------
Shard by (batch, head-group): core c handles batch b = c//4 and heads
h0 = (c%4)*4 .. h0+4 (16 heads total, 4 per core). Each core computes a
partial output  sum_h softmax(mask(q_h k_h^T / 8)) v_h @ Wo[h-rows, :]
for its 4 heads on its batch; the host sums the 4 partials per batch.

Sparsity: tokens are sorted by block id on the host (a pure data
permutation). After sorting, same-block attention is confined to a
+-1-tile band: a 128-query tile's valid keys always lie inside the fixed
384-wide window starting at tile clip(i-1, 0, 13) (block sizes <= 46 <<
128). So each [128, 2048] score row collapses to [128, 384].

Masking is folded into the score matmul itself: blockB is a one-hot
[N, 64], so  128 * sum_j (8*B[q,j]) * (16*B[k,j]*nodal[k])  adds exactly
+128 to same-block & key-unmasked pairs and 0 otherwise (one extra
K=64 accumulating matmul, shared PSUM). With exp(s + 128 - 144) the
invalid entries underflow to exactly 0, matching the reference's
additive -1e9 masking; 144 = 128 + 16 keeps exp in a safe range.

Softmax denominators come free from the Exp activation's accum_out;
normalization is a per-partition tensor_scalar multiply; the [q,k] ->
[k,q] transpose for the AV matmul goes through the PE transpose path.
"""

import numpy as np
from contextlib import ExitStack

import concourse.bass as bass
from concourse import bacc
import concourse.mybir as mybir
import concourse.tile as tile
from concourse.bass_utils import run_bass_kernel_spmd
from concourse.masks import make_identity

B = 2
N = 2048
UNITS = 1024
H = 16
HD = 64
NB = 64
NCORES = 8
HPC = 4            # heads per core
CPB = 4            # cores per batch
NT = N // 128      # 16 query tiles
WIN = 384          # key window (3 tiles)
NKT = UNITS // 128  # 8 contraction tiles for projections
EXP_BIAS = -144.0  # -(128 mask offset + 16 score headroom)

F32 = mybir.dt.float32
F32R = mybir.dt.float32r
BF16 = mybir.dt.bfloat16

# f32r (TF32-class, ~3e-4 rel) for the big matmuls; exact f32 for
# transposes + AV. Flip to False for an all-fp32 (exact) kernel.
USE_F32R = True
# stage granularity for debugging/tuning: which stages use f32r
F32R_STAGES = ("proj", "scores", "wo")
# AV matmul dtype: "f32" (exact, 4 cyc/row) or "bf16" (1 cyc/row)
AV_MODE = "bf16"
# pool depths (tunable): psum budget is 8 banks per phase
BUFS = {"xpool": 2, "ewpool": 2, "ewtpool": 2, "spool": 3, "stage": 3,
        "pp_qk": 4, "pp_v": 4, "pp_s": 2, "pp_tr": 2, "pp_av": 2, "pp_f": 1}

_CACHE = {}


def _build(use_f32r: bool, stages=None, av_mode=None):
    stages = set(stages if stages is not None else (F32R_STAGES if use_f32r else ()))
    av_mode = av_mode or AV_MODE
    pdt = F32R if "proj" in stages else F32    # xts/wq/wk/wv + qT/kT
    sdt = F32R if "scores" in stages else F32  # scores lhsT/rhs incl aug
    odt = F32R if "wo" in stages else F32      # outT + wo
    avdt = BF16 if av_mode == "bf16" else F32  # v + transposed weights
    mmdt = pdt
    nc = bacc.Bacc("TRN2", target_bir_lowering=False, debug=False)

    xts = nc.dram_tensor("xts", [UNITS, N], mmdt, kind="ExternalInput")
    wq = nc.dram_tensor("wq", [128, NKT, HPC * HD], mmdt, kind="ExternalInput")
    wk = nc.dram_tensor("wk", [128, NKT, HPC * HD], mmdt, kind="ExternalInput")
    wv = nc.dram_tensor("wv", [128, NKT, HPC * HD], mmdt, kind="ExternalInput")
    wo = nc.dram_tensor("wo", [128, 2, UNITS], odt, kind="ExternalInput")
    augq = nc.dram_tensor("augq", [NB, N], sdt, kind="ExternalInput")
    augk = nc.dram_tensor("augk", [NB, N], sdt, kind="ExternalInput")
    out = nc.dram_tensor("out", [N, UNITS], F32, kind="ExternalOutput")

    with tile.TileContext(nc) as tc, ExitStack() as ctx:
        singles = ctx.enter_context(tc.tile_pool(name="singles", bufs=1))
        persist = ctx.enter_context(tc.tile_pool(name="persist", bufs=1))

        # ---- constants / persistent tensors ----
        # DMA order matters: the first projection matmuls need wq/wk and
        # the first x slices; wv a bit later; wo and the mask-aug rows
        # only after the whole projection phase. Emit in that order so
        # the DMA queues drain the critical path first.
        wq_sb = persist.tile([128, NKT, HPC * HD], mmdt, tag="wq_sb")
        nc.sync.dma_start(wq_sb[:], wq.ap())
        wk_sb = persist.tile([128, NKT, HPC * HD], mmdt, tag="wk_sb")
        nc.sync.dma_start(wk_sb[:], wk.ap())
        wv_sb = persist.tile([128, NKT, HPC * HD], mmdt, tag="wv_sb")
        wo_sb = persist.tile([128, 2, UNITS], odt, tag="wo_sb")

        ident = singles.tile([128, 128], F32)
        make_identity(nc, ident[:])
        if avdt != F32:
            ident_av = singles.tile([128, 128], avdt)
            nc.vector.tensor_copy(ident_av[:], ident[:])
        else:
            ident_av = ident
        bias_t = singles.tile([128, 1], F32)
        nc.vector.memset(bias_t[:], EXP_BIAS)

        # Per-head base-partition-0 layout: f32r matmul operands at
        # base_partition 64 fault the exec unit (HW erratum) unless the
        # tile came straight from a DRAM DMA, so heads get their own
        # [*, h, :] slices along the free dim instead of partition 64..127.
        # Rows 0..63 hold qT/kT (hd=64); rows 64..127 hold the mask-aug
        # one-hot rows (shared across heads, DMA-replicated), making the
        # score matmul a single K=128 pass that computes q.k/8 + mask.
        qT_sb = persist.tile([128, HPC, N], sdt, tag="qT_sb")
        kT_sb = persist.tile([128, HPC, N], sdt, tag="kT_sb")
        v_sb = persist.tile([128, NT, HPC * HD], avdt, tag="v_sb")
        outT_sb = persist.tile([128, 2, N], odt, tag="outT_sb")

        # ---- projections (4 column passes over N) ----
        xpool = ctx.enter_context(tc.tile_pool(name="xpool", bufs=BUFS["xpool"]))
        with tc.tile_pool(name="pp_qk", bufs=BUFS["pp_qk"], space="PSUM") as pp_qk, \
             tc.tile_pool(name="pp_v", bufs=BUFS["pp_v"], space="PSUM") as pp_v:
            for fc in range(4):
                if fc == 1:
                    # aug mask rows: one small DMA each, replicated
                    # per-head by the otherwise-idle GPSIMD engine
                    nc.sync.dma_start(qT_sb[64:128, 0, :], augq.ap())
                    nc.sync.dma_start(kT_sb[64:128, 0, :], augk.ap())
                    for h in range(1, HPC):
                        nc.vector.tensor_copy(qT_sb[64:128, h, :],
                                              qT_sb[64:128, 0, :])
                        nc.vector.tensor_copy(kT_sb[64:128, h, :],
                                              kT_sb[64:128, 0, :])
                elif fc == 2:
                    nc.sync.dma_start(wo_sb[:], wo.ap())
                xt = xpool.tile([128, NKT, 512], mmdt, tag="xt")
                for kt in range(NKT):
                    nc.sync.dma_start(
                        xt[:, kt, :],
                        xts.ap()[kt * 128:(kt + 1) * 128, fc * 512:(fc + 1) * 512])
                for dst, w_sb in ((qT_sb, wq_sb), (kT_sb, wk_sb)):
                    for m in range(2):
                        ps = pp_qk.tile([128, 512], F32, tag="ps_qk")
                        for kt in range(NKT):
                            nc.tensor.matmul(
                                ps[:], w_sb[:, kt, m * 128:(m + 1) * 128],
                                xt[:, kt, :],
                                start=(kt == 0), stop=(kt == NKT - 1))
                        nc.vector.tensor_copy(
                            dst[0:64, 2 * m, fc * 512:(fc + 1) * 512],
                            ps[0:64, :])
                        nc.vector.tensor_copy(
                            dst[0:64, 2 * m + 1, fc * 512:(fc + 1) * 512],
                            ps[64:128, :])
                if fc == 0:
                    nc.sync.dma_start(wv_sb[:], wv.ap())
                for qi in range(4):
                    ps = pp_v.tile([128, HPC * HD], F32, tag="ps_v")
                    for kt in range(NKT):
                        nc.tensor.matmul(
                            ps[:], xt[:, kt, qi * 128:(qi + 1) * 128],
                            wv_sb[:, kt, :],
                            start=(kt == 0), stop=(kt == NKT - 1))
                    nc.vector.tensor_copy(v_sb[:, fc * 4 + qi, :], ps[:])

        # ---- attention + output projection ----
        ewpool = ctx.enter_context(tc.tile_pool(name="ewpool", bufs=BUFS["ewpool"]))
        ewtpool = ctx.enter_context(tc.tile_pool(name="ewtpool", bufs=BUFS["ewtpool"]))
        spool = ctx.enter_context(tc.tile_pool(name="spool", bufs=BUFS["spool"]))
        stage = ctx.enter_context(tc.tile_pool(name="stage", bufs=BUFS["stage"]))
        pp_s = ctx.enter_context(tc.tile_pool(name="pp_s", bufs=BUFS["pp_s"], space="PSUM"))
        pp_tr = ctx.enter_context(tc.tile_pool(name="pp_tr", bufs=BUFS["pp_tr"], space="PSUM"))
        pp_av = ctx.enter_context(tc.tile_pool(name="pp_av", bufs=BUFS["pp_av"], space="PSUM"))
        pp_f = ctx.enter_context(tc.tile_pool(name="pp_f", bufs=BUFS["pp_f"], space="PSUM"))

        for i in range(NT):
            t0 = min(max(i - 1, 0), NT - 3)
            qs = slice(i * 128, (i + 1) * 128)
            ks = slice(t0 * 128, t0 * 128 + WIN)
            stats = spool.tile([128, HPC], F32, tag="stats")
            ews = []
            for h in range(HPC):
                mt, po = h // 2, (h % 2) * 64
                s_ps = pp_s.tile([128, WIN], F32, tag="s_ps")
                nc.tensor.matmul(s_ps[:], qT_sb[:, h, qs],
                                 kT_sb[:, h, ks],
                                 start=True, stop=True)
                ew = ewpool.tile([128, WIN], avdt, tag=f"ew{h}")
                nc.scalar.activation(ew[:], s_ps[:],
                                     mybir.ActivationFunctionType.Exp,
                                     bias=bias_t[:], scale=1.0,
                                     accum_out=stats[:, h:h + 1])
                ews.append(ew)
            r_t = spool.tile([128, HPC], F32, tag="r_t")
            nc.vector.reciprocal(r_t[:], stats[:])
            for h in range(HPC):
                mt, po = h // 2, (h % 2) * 64
                ew = ews[h]
                nc.vector.tensor_scalar_mul(ew[:], ew[:], r_t[:, h:h + 1])
                tr_ps = pp_tr.tile([128, WIN], avdt, tag="tr_ps")
                for j in range(3):
                    nc.tensor.transpose(
                        tr_ps[:, j * 128:(j + 1) * 128],
                        ew[:, j * 128:(j + 1) * 128], ident_av[:])
                ewt = ewtpool.tile([128, WIN], avdt, tag="ewt")
                nc.vector.tensor_copy(ewt[:], tr_ps[:])
                av_ps = pp_av.tile([64, 128], F32, tag="av_ps")
                for j in range(3):
                    nc.tensor.matmul(av_ps[:], v_sb[:, t0 + j, h * HD:(h + 1) * HD],
                                     ewt[:, j * 128:(j + 1) * 128],
                                     start=(j == 0), stop=(j == 2))
                nc.vector.tensor_copy(outT_sb[po:po + 64, mt, qs], av_ps[:])
            # output projection for this query tile
            f_ps = pp_f.tile([128, UNITS], F32, tag="f_ps")
            for fc2 in range(2):
                for mt in range(2):
                    nc.tensor.matmul(
                        f_ps[:, fc2 * 512:(fc2 + 1) * 512],
                        outT_sb[:, mt, qs],
                        wo_sb[:, mt, fc2 * 512:(fc2 + 1) * 512],
                        start=(mt == 0), stop=(mt == 1))
            st = stage.tile([128, UNITS], F32, tag="st")
            nc.scalar.copy(st[:], f_ps[:])
            nc.sync.dma_start(out.ap()[qs, :], st[:])

    nc.compile()
    return nc


def _get_nc():
    key = (USE_F32R, tuple(F32R_STAGES) if USE_F32R else (), AV_MODE)
    if key not in _CACHE:
        _CACHE[key] = _build(USE_F32R, F32R_STAGES if USE_F32R else (),
                             AV_MODE)
    return _CACHE[key]


def kernel(x, blockB, NodalMask, Wq, Wk, Wv, Wo):
    x = np.asarray(x, dtype=np.float32)
    blockB = np.asarray(blockB, dtype=np.float32)
    NodalMask = np.asarray(NodalMask, dtype=np.float32)
    Wq = np.asarray(Wq, dtype=np.float32)
    Wk = np.asarray(Wk, dtype=np.float32)
    Wv = np.asarray(Wv, dtype=np.float32)
    Wo = np.asarray(Wo, dtype=np.float32)

    # host-side data prep: sort tokens by block id per batch
    perms = []
    in_maps = []
    for c in range(NCORES):
        b, hg = c // CPB, c % CPB
        if hg == 0:
            bids = blockB[b].argmax(-1)
            perm = np.argsort(bids, kind="stable")
            perms.append(perm)
        perm = perms[b]
        h0 = hg * HPC
        cols = slice(h0 * HD, (h0 + HPC) * HD)
        xs = x[b][perm]                                # [N, UNITS] sorted
        wq_h = (Wq[:, cols] * 0.125).reshape(NKT, 128, HPC * HD).transpose(1, 0, 2)
        wk_h = Wk[:, cols].reshape(NKT, 128, HPC * HD).transpose(1, 0, 2)
        wv_h = Wv[:, cols].reshape(NKT, 128, HPC * HD).transpose(1, 0, 2)
        wo_h = Wo[cols, :].reshape(2, 128, UNITS).transpose(1, 0, 2)
        bb = blockB[b][perm]                           # [N, NB] one-hot sorted
        augq_h = np.ascontiguousarray(8.0 * bb.T)
        augk_h = np.ascontiguousarray(
            16.0 * (bb * NodalMask[b][perm][:, None]).T)
        in_maps.append({
            "xts": np.ascontiguousarray(xs.T),
            "wq": np.ascontiguousarray(wq_h),
            "wk": np.ascontiguousarray(wk_h),
            "wv": np.ascontiguousarray(wv_h),
            "wo": np.ascontiguousarray(wo_h),
            "augq": augq_h,
            "augk": augk_h,
        })

    global _last_in_maps
    _last_in_maps = in_maps
    nc = _get_nc()
    res = run_bass_kernel_spmd(nc, in_maps, core_ids=list(range(NCORES)))

    result = np.empty((B, N, UNITS), dtype=np.float32)
    for b in range(B):
        acc = res.results[b * CPB]["out"].astype(np.float32)
        for hg in range(1, CPB):
            acc = acc + res.results[b * CPB + hg]["out"]
        result[b][perms[b]] = acc
    return result



# revision 3
# speedup vs baseline: 1.2564x; 1.2564x over previous
"""Block-sparse self-attention Trainium2 kernel (8 NeuronCores).

Strategy
--------
Shard by (batch, head-group): core c handles batch b = c//4 and heads
(c%4)*4 .. +4. Each core computes a partial output
sum_h softmax(mask(q_h k_h^T / 8)) v_h @ Wo[h-rows, :] for its 4 heads;
the host sums the 4 partials per batch.

Token layout (per batch, host-side): [valid tokens sorted by block |
pad to NVP | invalid tokens sorted by block | pad to NQ]. Keys are the
first NVP positions only (valid tokens + masked pad), so the K/V
projections run on ~NVP=1152 columns instead of 2048 and each query
tile's same-block keys lie in a short contiguous window of the key
prefix. Per-tile window starts/widths (128-aligned, 256 or 384 wide)
are derived from the actual block assignment at build time and baked
into the instruction stream; the module cache is keyed on them.

Masking is folded into the score matmul: blockB is one-hot [N, 64], so
rows 64..127 of the qT/kT operands hold 8*onehot(q) and
16*onehot(k)(valid-only), making the K=128 score matmul compute
q.k/8 + 128*[same block & key valid]. exp(s + 128 - 144) then
underflows invalid pairs to exactly 0 (matches additive -1e9 masking).

All inputs stream in as bf16 (halves DMA, which is near-critical);
score/AV/Wo matmuls run bf16 at 1 cycle/row. PSUM->SBUF traffic is
spread across DVE/Act/Pool: softmax normalization (tensor_scalar by
1/denom from the Exp accumulator) runs on the otherwise-idle Pool
engine, q/k head-splitting goes through a bf16 staging tile (Pool/Act
do the SBUF->SBUF splits), transposes and AV outputs are pair-packed so
one copy moves two heads. Output is written bf16 and summed on host.
"""

import numpy as np
import ml_dtypes
from contextlib import ExitStack

import concourse.bass as bass
from concourse import bacc
import concourse.mybir as mybir
import concourse.tile as tile
from concourse.bass_utils import run_bass_kernel_spmd
from concourse.masks import make_identity

B = 2
N = 2048
UNITS = 1024
H = 16
HD = 64
NB = 64
NCORES = 8
HPC = 4            # heads per core
CPB = 4            # cores per batch
NKT = UNITS // 128  # 8 contraction tiles for projections
EXP_BIAS = -144.0  # -(128 mask offset + 16 score headroom)

F32 = mybir.dt.float32
BF16 = mybir.dt.bfloat16
BF16NP = ml_dtypes.bfloat16

_CACHE = {}
_LAST_NC = None


def _build(nq: int, nvp: int, windows: tuple):
    """windows: per query-tile (t0, w) with t0 a key-tile index and w in
    {256, 384}; window keys are [t0*128, t0*128+w) of the NVP prefix."""
    ntq = nq // 128
    ntk = nvp // 128
    assert len(windows) == ntq
    nc = bacc.Bacc("TRN2", target_bir_lowering=False, debug=False)

    xts = nc.dram_tensor("xts", [128, NKT, nq], BF16, kind="ExternalInput")
    wq = nc.dram_tensor("wq", [128, NKT, HPC * HD], BF16, kind="ExternalInput")
    wk = nc.dram_tensor("wk", [128, NKT, HPC * HD], BF16, kind="ExternalInput")
    wv = nc.dram_tensor("wv", [128, NKT, HPC * HD], BF16, kind="ExternalInput")
    wo = nc.dram_tensor("wo", [128, 2, UNITS], BF16, kind="ExternalInput")
    augq = nc.dram_tensor("augq", [NB, HPC, nq], BF16, kind="ExternalInput")
    augk = nc.dram_tensor("augk", [NB, HPC, nvp], BF16, kind="ExternalInput")
    out = nc.dram_tensor("out", [nq, UNITS], BF16, kind="ExternalOutput")

    # query-column chunks for the projection sweep
    chunks = []
    c0 = 0
    while c0 < nq:
        cw = min(512, nq - c0)
        chunks.append((c0, cw))
        c0 += cw

    # attention tile i is emitted once projections cover its query tile
    # and its key window (kT/v live in the NVP prefix)
    def attn_ready(i, cols):
        t0, w = windows[i]
        return (i + 1) * 128 <= cols and t0 * 128 + w <= cols

    with tile.TileContext(nc) as tc, ExitStack() as ctx:
        singles = ctx.enter_context(tc.tile_pool(name="singles", bufs=1))
        persist = ctx.enter_context(tc.tile_pool(name="persist", bufs=1))

        # ---- persistent tensors; DMA emission order = queue drain order
        wq_sb = persist.tile([128, NKT, HPC * HD], BF16, tag="wq_sb")
        nc.sync.dma_start(wq_sb[:], wq.ap())
        wk_sb = persist.tile([128, NKT, HPC * HD], BF16, tag="wk_sb")
        nc.sync.dma_start(wk_sb[:], wk.ap())
        x_sb = persist.tile([128, NKT, nq], BF16, tag="x_sb")
        # first projection chunk's x first, then the rest per kt
        for kt in range(NKT):
            nc.sync.dma_start(x_sb[:, kt, 0:512], xts.ap()[:, kt, 0:512])
        wv_sb = persist.tile([128, NKT, HPC * HD], BF16, tag="wv_sb")
        nc.sync.dma_start(wv_sb[:], wv.ap())
        for kt in range(NKT):
            nc.sync.dma_start(x_sb[:, kt, 512:1024], xts.ap()[:, kt, 512:1024])

        qT_sb = persist.tile([128, HPC, nq], BF16, tag="qT_sb")
        kT_sb = persist.tile([128, HPC, nvp], BF16, tag="kT_sb")
        v_sb = persist.tile([128, ntk, HPC * HD], BF16, tag="v_sb")
        wo_sb = persist.tile([128, 2, UNITS], BF16, tag="wo_sb")
        # mask-aug rows, host-replicated per head: one DMA each
        nc.sync.dma_start(qT_sb[64:128, :, :], augq.ap())
        nc.sync.dma_start(kT_sb[64:128, :, :], augk.ap())
        nc.sync.dma_start(wo_sb[:], wo.ap())
        for kt in range(NKT):
            nc.sync.dma_start(x_sb[:, kt, 1024:nq], xts.ap()[:, kt, 1024:nq])

        ident = singles.tile([128, 128], F32)
        make_identity(nc, ident[:])
        ident_bf = singles.tile([128, 128], BF16)
        nc.vector.tensor_copy(ident_bf[:], ident[:])
        bias_t = singles.tile([128, 1], F32)
        nc.vector.memset(bias_t[:], EXP_BIAS)

        stage = ctx.enter_context(tc.tile_pool(name="stage", bufs=3))
        ewpool = ctx.enter_context(tc.tile_pool(name="ewpool", bufs=2))
        ewtpool = ctx.enter_context(tc.tile_pool(name="ewtpool", bufs=2))
        spool = ctx.enter_context(tc.tile_pool(name="spool", bufs=3))
        otpool = ctx.enter_context(tc.tile_pool(name="otpool", bufs=2))
        stpool = ctx.enter_context(tc.tile_pool(name="stpool", bufs=3))
        pp_proj = ctx.enter_context(
            tc.tile_pool(name="pp_proj", bufs=2, space="PSUM"))
        pp_s = ctx.enter_context(tc.tile_pool(name="pp_s", bufs=2, space="PSUM"))
        pp_tr = ctx.enter_context(tc.tile_pool(name="pp_tr", bufs=2, space="PSUM"))
        pp_av = ctx.enter_context(tc.tile_pool(name="pp_av", bufs=1, space="PSUM"))
        pp_f = ctx.enter_context(tc.tile_pool(name="pp_f", bufs=1, space="PSUM"))

        def emit_attn_tile(i):
            t0, w = windows[i]
            nch = w // 128
            qs = slice(i * 128, (i + 1) * 128)
            ks = slice(t0 * 128, t0 * 128 + w)
            stats = spool.tile([128, HPC], F32, tag="stats")
            ews = []
            for h in range(HPC):
                s_ps = pp_s.tile([128, 384], F32, tag="s_ps")
                nc.tensor.matmul(s_ps[:, 0:w], qT_sb[:, h, qs],
                                 kT_sb[:, h, ks], start=True, stop=True)
                ew = ewpool.tile([128, 384], BF16, tag=f"ew{h}")
                nc.scalar.activation(ew[:, 0:w], s_ps[:, 0:w],
                                     mybir.ActivationFunctionType.Exp,
                                     bias=bias_t[:], scale=1.0,
                                     accum_out=stats[:, h:h + 1])
                ews.append(ew)
            r_t = spool.tile([128, HPC], F32, tag="r_t")
            nc.vector.reciprocal(r_t[:], stats[:])
            av2 = pp_av.tile([128, 2, 128], F32, tag="av2")
            for hp in range(2):
                tr2 = pp_tr.tile([128, 2, 384], BF16, tag="tr2")
                for s in range(2):
                    h = 2 * hp + s
                    nc.gpsimd.tensor_scalar_mul(ews[h][:, 0:w], ews[h][:, 0:w],
                                                r_t[:, h:h + 1])
                    for j in range(nch):
                        nc.tensor.transpose(
                            tr2[:, s, j * 128:(j + 1) * 128],
                            ews[h][:, j * 128:(j + 1) * 128], ident_bf[:])
                ewt = ewtpool.tile([128, 2, 384], BF16, tag="ewt")
                nc.vector.tensor_copy(ewt[:, :, 0:w], tr2[:, :, 0:w])
                for s in range(2):
                    h = 2 * hp + s
                    for j in range(nch):
                        nc.tensor.matmul(
                            av2[s * 64:(s + 1) * 64, hp, :],
                            v_sb[:, t0 + j, h * HD:(h + 1) * HD],
                            ewt[:, s, j * 128:(j + 1) * 128],
                            start=(j == 0), stop=(j == nch - 1))
            outT = otpool.tile([128, 2, 128], BF16, tag="outT")
            nc.vector.tensor_copy(outT[:], av2[:])
            st = stpool.tile([128, UNITS], BF16, tag="st")
            for fc2 in range(2):
                f_ps = pp_f.tile([128, 512], F32, tag="f_ps")
                for mt in range(2):
                    nc.tensor.matmul(
                        f_ps[:], outT[:, mt, :],
                        wo_sb[:, mt, fc2 * 512:(fc2 + 1) * 512],
                        start=(mt == 0), stop=(mt == 1))
                nc.vector.tensor_copy(st[:, fc2 * 512:(fc2 + 1) * 512], f_ps[:])
            nc.sync.dma_start(out.ap()[qs, :], st[:])

        # ---- projection sweep with attention tiles interleaved ----
        attn_done = 0
        for (c0, cw) in chunks:
            cols_k = min(max(nvp - c0, 0), cw)  # kT/v columns in this chunk
            for dst, w_sb, cw_d in ((qT_sb, wq_sb, cw), (kT_sb, wk_sb, cols_k)):
                if cw_d == 0:
                    continue
                for m in range(2):
                    ps = pp_proj.tile([128, 512], F32, tag="ps")
                    for kt in range(NKT):
                        nc.tensor.matmul(
                            ps[:, 0:cw_d], w_sb[:, kt, m * 128:(m + 1) * 128],
                            x_sb[:, kt, c0:c0 + cw_d],
                            start=(kt == 0), stop=(kt == NKT - 1))
                    stg = stage.tile([128, 512], BF16, tag="stg")
                    nc.vector.tensor_copy(stg[:, 0:cw_d], ps[:, 0:cw_d])
                    # head split: SBUF->SBUF on Pool (q) / Act (k)
                    eng = nc.gpsimd if dst is qT_sb else nc.scalar
                    for s in range(2):
                        h = 2 * m + s
                        if eng is nc.scalar:
                            eng.copy(dst[0:64, h, c0:c0 + cw_d],
                                     stg[s * 64:(s + 1) * 64, 0:cw_d])
                        else:
                            eng.tensor_copy(dst[0:64, h, c0:c0 + cw_d],
                                            stg[s * 64:(s + 1) * 64, 0:cw_d])
            for qi in range(cols_k // 128):
                ps = pp_proj.tile([128, 512], F32, tag="ps")
                for kt in range(NKT):
                    nc.tensor.matmul(
                        ps[:, 0:HPC * HD],
                        x_sb[:, kt, c0 + qi * 128:c0 + (qi + 1) * 128],
                        wv_sb[:, kt, :], start=(kt == 0), stop=(kt == NKT - 1))
                nc.vector.tensor_copy(v_sb[:, c0 // 128 + qi, :],
                                      ps[:, 0:HPC * HD])
            cols = c0 + cw
            while attn_done < ntq and attn_ready(attn_done, cols):
                emit_attn_tile(attn_done)
                attn_done += 1
        while attn_done < ntq:
            emit_attn_tile(attn_done)
            attn_done += 1

    nc.compile()
    return nc


def _get_nc(key=None):
    global _LAST_NC
    if key is None:
        return _LAST_NC
    if key not in _CACHE:
        _CACHE[key] = _build(*key)
    _LAST_NC = _CACHE[key]
    return _CACHE[key]


def _plan(blockB, NodalMask):
    """Token layout + per-tile key windows, shared across both batches."""
    bids = [np.argmax(blockB[b], -1) for b in range(B)]
    nodal = [NodalMask[b] != 0 for b in range(B)]
    nv = [int(nodal[b].sum()) for b in range(B)]
    nvp = -(-max(nv) // 128) * 128
    ni_max = max(N - v for v in nv)
    nq = nvp + (-(-ni_max // 128) * 128)

    perms = []      # per batch: token index for each real layout position
    pos = []        # per batch: layout position of each token
    vstart = []     # per batch: [NB+1] cumulative valid-key starts
    for b in range(B):
        order_v = np.argsort(np.where(nodal[b], bids[b], NB + 1), kind="stable")
        perm_v = order_v[: nv[b]]                     # valid by block
        order_i = np.argsort(np.where(~nodal[b], bids[b], NB + 1),
                             kind="stable")
        perm_i = order_i[: N - nv[b]]                 # invalid by block
        p = np.full(N, -1, np.int64)
        p[perm_v] = np.arange(nv[b])
        p[perm_i] = nvp + np.arange(N - nv[b])
        perms.append((perm_v, perm_i))
        pos.append(p)
        counts = np.bincount(bids[b][nodal[b]], minlength=NB)
        vstart.append(np.concatenate([[0], np.cumsum(counts)]))

    windows = []
    for i in range(nq // 128):
        lo, hi = nvp, 0
        for b in range(B):
            in_tile = (pos[b] >= i * 128) & (pos[b] < (i + 1) * 128)
            if not in_tile.any():
                continue
            blk = bids[b][in_tile]
            lo = min(lo, int(vstart[b][blk.min()]))
            hi = max(hi, int(vstart[b][blk.max() + 1]))
        if hi <= lo:
            windows.append((0, 256))
            continue
        t0 = lo // 128
        w = 256 if hi <= t0 * 128 + 256 else 384
        t0 = min(t0, nvp // 128 - w // 128)
        assert lo >= t0 * 128 and hi <= t0 * 128 + w, (i, lo, hi, t0, w)
        windows.append((t0, w))
    return nq, nvp, tuple(windows), perms, pos


def kernel(x, blockB, NodalMask, Wq, Wk, Wv, Wo):
    x = np.asarray(x, dtype=np.float32)
    blockB = np.asarray(blockB, dtype=np.float32)
    NodalMask = np.asarray(NodalMask, dtype=np.float32)
    Wq = np.asarray(Wq, dtype=np.float32)
    Wk = np.asarray(Wk, dtype=np.float32)
    Wv = np.asarray(Wv, dtype=np.float32)
    Wo = np.asarray(Wo, dtype=np.float32)

    nq, nvp, windows, perms, pos = _plan(blockB, NodalMask)

    in_maps = []
    batch_data = []
    for b in range(B):
        perm_v, perm_i = perms[b]
        xs = np.zeros((nq, UNITS), np.float32)
        xs[: len(perm_v)] = x[b][perm_v]
        xs[nvp: nvp + len(perm_i)] = x[b][perm_i]
        xts = np.ascontiguousarray(
            xs.T.reshape(NKT, 128, nq).transpose(1, 0, 2)).astype(BF16NP)
        aq = np.zeros((NB, nq), np.float32)
        aq[:, : len(perm_v)] = 8.0 * blockB[b][perm_v].T
        aq[:, nvp: nvp + len(perm_i)] = 8.0 * blockB[b][perm_i].T
        ak = np.zeros((NB, nvp), np.float32)
        ak[:, : len(perm_v)] = 16.0 * blockB[b][perm_v].T
        augq = np.ascontiguousarray(
            np.broadcast_to(aq[:, None, :], (NB, HPC, nq))).astype(BF16NP)
        augk = np.ascontiguousarray(
            np.broadcast_to(ak[:, None, :], (NB, HPC, nvp))).astype(BF16NP)
        batch_data.append((xts, augq, augk))

    for c in range(NCORES):
        b, hg = c // CPB, c % CPB
        xts, augq, augk = batch_data[b]
        cols = slice(hg * HPC * HD, (hg + 1) * HPC * HD)
        wq_h = (Wq[:, cols] * 0.125).reshape(NKT, 128, HPC * HD)
        wk_h = Wk[:, cols].reshape(NKT, 128, HPC * HD)
        wv_h = Wv[:, cols].reshape(NKT, 128, HPC * HD)
        wo_h = Wo[cols, :].reshape(2, 128, UNITS)
        in_maps.append({
            "xts": xts,
            "wq": np.ascontiguousarray(wq_h.transpose(1, 0, 2)).astype(BF16NP),
            "wk": np.ascontiguousarray(wk_h.transpose(1, 0, 2)).astype(BF16NP),
            "wv": np.ascontiguousarray(wv_h.transpose(1, 0, 2)).astype(BF16NP),
            "wo": np.ascontiguousarray(wo_h.transpose(1, 0, 2)).astype(BF16NP),
            "augq": augq,
            "augk": augk,
        })

    nc = _get_nc((nq, nvp, windows))
    res = run_bass_kernel_spmd(nc, in_maps, core_ids=list(range(NCORES)))

    result = np.empty((B, N, UNITS), dtype=np.float32)
    for b in range(B):
        acc = res.results[b * CPB]["out"].astype(np.float32)
        for hg in range(1, CPB):
            acc = acc + res.results[b * CPB + hg]["out"].astype(np.float32)
        perm_v, perm_i = perms[b]
        result[b][perm_v] = acc[: len(perm_v)]
        result[b][perm_i] = acc[nvp: nvp + len(perm_i)]
    return result
